# revision 1
# baseline (speedup 1.0000x reference)
"""Trainium2 Bass kernel for nn_CAModel (neural cellular automaton step).

Strategy (pure data parallel, B=32 -> 4 images per core x 8 cores):
- Host pre-transposes to channel-major padded layout; device partition p =
  (img_local, half, channel) = 4*2*16 = 128.  All spatial shifts become
  free-dim offsets (row pitch 130, zero ring).
- Depthwise sobel conv as separable shifted adds on VectorE in bf16.
- fc0 as 3 accumulating K=32 matmuls per group (zero-padded weights per
  group parity), 4 partition strips run concurrently on the PE sub-arrays.
- relu PSUM->SBUF copy split between ScalarE and VectorE, bf16 out.
- fc1 as K=128 -> M=32 matmul pairs accumulating both group parities.
- residual + update mask + alive mask (3x3 maxpool in a strip layout,
  scatter/broadcast via SBUF-SBUF DMA) on VectorE.
"""

import dataclasses
import numpy as np
import ml_dtypes

import concourse.bass as bass
import concourse.tile as tile
from concourse import mybir, bass_utils
import bass_rust

F32 = mybir.dt.float32
BF16 = mybir.dt.bfloat16
ALU = mybir.AluOpType
ACTF = mybir.ActivationFunctionType

N_CORES = 8
B, H, W, C = 32, 128, 128, 16
HID = 128
IMGS = B // N_CORES          # 4 images per core
GRP = IMGS * 2               # 8 (img, half) groups per core
PW = W + 2                   # padded row pitch 130
PR = H // 2 + 2              # padded rows per half 66
NPAD = PR * PW               # 8580
NPIX = (H // 2) * W          # 8192 interior pixels per group
CHUNK = 1024                 # pixels per MLP chunk (8 interior rows)
NCHUNK = NPIX // CHUNK       # 8
X2G = 128                    # guard elems around x2 free dim
RELU_PATTERN = (True, True, False)  # True -> ScalarE


def _split_multiwaits(nc):
    """walrus in this env only supports one sem-wait per instruction."""
    n = 0
    for f in nc.m.functions:
        for bb in f.blocks:
            out = []
            changed = False
            for inst in bb.instructions:
                si = inst.sync_info
                if si is not None and len(si.on_wait) > 1:
                    waits = list(si.on_wait)
                    for k, w in enumerate(waits[:-1]):
                        nop = mybir.InstNoOp(
                            name=f"{inst.name}_ws{k}",
                            sync_info=mybir.SyncInfo(on_wait=[w], on_update=[]),
                            bass_nofuse=True,
                            engine=inst.engine,
                        )
                        nc.register_instruction(nop, overwrite=True)
                        out.append(nop)
                        n += 1
                    inst.sync_info = mybir.SyncInfo(
                        on_wait=[waits[-1]], on_update=list(si.on_update)
                    )
                    changed = True
                out.append(inst)
            if changed:
                bb.instructions[:] = out
    return n


def _mk_ap(ap, offset, dims):
    return dataclasses.replace(ap, offset=offset, ap=[list(d) for d in dims])


def build_program():
    nc = bass.Bass()

    xpad_d = nc.dram_tensor("xpad", [128, NPAD], F32, kind="ExternalInput").ap()
    u16_d = nc.dram_tensor("u16", [128, NPIX], BF16, kind="ExternalInput").ap()
    astrip_d = nc.dram_tensor("astrip", [128, 780], F32, kind="ExternalInput").ap()
    w0_d = {}
    for feat in ("id", "dx", "dy"):
        for gg in range(2):
            w0_d[(feat, gg)] = nc.dram_tensor(
                f"w0{feat}{gg}", [128, 128], BF16, kind="ExternalInput"
            ).ap()
    w1_d = [
        nc.dram_tensor(f"w1{gg}", [128, 32], BF16, kind="ExternalInput").ap()
        for gg in range(2)
    ]
    sel_d = nc.dram_tensor("sel", [128, 2048], BF16, kind="ExternalInput").ap()
    out_d = nc.dram_tensor("out", [128, NPIX], F32, kind="ExternalOutput").ap()

    with tile.TileContext(nc) as tc:
        with tc.tile_pool(name="persist", bufs=1) as pp:
            # --- persistent tiles ---
            xpad = pp.tile([128, NPAD], F32, tag="xpad")
            xb = pp.tile([128, NPAD + 4], BF16, tag="xb")        # data at +2
            ydx = pp.tile([128, 64 * PW], BF16, tag="ydx")
            ydy = pp.tile([128, 64 * PW], BF16, tag="ydy")
            astrip = pp.tile([128, 780], F32, tag="astrip")
            a2strip = pp.tile([128, 780], F32, tag="a2strip")
            selt = pp.tile([128, 2048], BF16, tag="selt")
            nc.sync.dma_start(out=selt[:, :], in_=sel_d)
            w0t = {k: pp.tile([128, 128], BF16, tag=f"w0{k[0]}{k[1]}", name=f"w0t{k[0]}{k[1]}") for k in w0_d}
            w1t = [pp.tile([128, 32], BF16, tag=f"w1{gg}", name=f"w1t{gg}") for gg in range(2)]

            # --- input DMAs ---
            for k in w0_d:
                nc.sync.dma_start(out=w0t[k][:, :], in_=w0_d[k])
            for gg in range(2):
                nc.sync.dma_start(out=w1t[gg][:, :], in_=w1_d[gg])
            CAST_BANDS = [(0, 18), (18, 34), (34, 50), (50, 66)]
            for lo, hi in CAST_BANDS:
                nc.sync.dma_start(
                    out=xpad[:, lo * PW : hi * PW], in_=xpad_d[:, lo * PW : hi * PW]
                )
            nc.sync.dma_start(out=astrip[:, :], in_=astrip_d)

            # --- cast to bf16 (ScalarE, per band) ---
            for lo, hi in CAST_BANDS:
                nc.scalar.activation(
                    out=xb[:, 2 + lo * PW : 2 + hi * PW],
                    in_=xpad[:, lo * PW : hi * PW],
                    func=ACTF.Copy,
                )

            # --- conv (VectorE, bf16) ---
            with tc.tile_pool(name="convp", bufs=1) as cp:
                xx2 = cp.tile([128, NPAD], BF16, tag="xx2")
                t_a = cp.tile([128, NPAD + 4], BF16, tag="c1", name="sv_t")

                tv = cp.tile([128, NPAD + 4], BF16, tag="c2", name="tv_t")
                t_b = cp.tile([128, NPAD + 4], BF16, tag="c1b", name="sh_t")
                th = cp.tile([128, NPAD + 4], BF16, tag="c2b", name="th_t")
                # per band: xx2/sh over padded-row range [lo,hi);
                # sv/tv/ydx/ydy over interior rows [max(lo,1) .. min(hi,65))
                for bi, (lo, hi) in enumerate(CAST_BANDS):
                    nc.vector.tensor_scalar_mul(
                        xx2[:, lo * PW : hi * PW], xb[:, 2 + lo * PW : 2 + hi * PW], 2.0
                    )
                    nc.vector.tensor_tensor(
                        out=t_b[:, 2 + lo * PW : 2 + hi * PW],
                        in0=xb[:, 1 + lo * PW : 1 + hi * PW],
                        in1=xb[:, 3 + lo * PW : 3 + hi * PW],
                        op=ALU.add,
                    )
                    nc.vector.tensor_tensor(
                        out=th[:, 2 + lo * PW : 2 + hi * PW],
                        in0=t_b[:, 2 + lo * PW : 2 + hi * PW],
                        in1=xx2[:, lo * PW : hi * PW],
                        op=ALU.add,
                    )
                    if bi == 0:
                        continue
                    # dx/dy outputs for rows covered by casts emitted so far
                    lo, hi = CAST_BANDS[bi - 1]
                    rl, rh_ = max(lo, 1), min(hi, 65)
                    nc.vector.tensor_tensor(
                        out=t_a[:, 2 + rl * PW : 2 + rh_ * PW],
                        in0=xb[:, 2 + (rl - 1) * PW : 2 + (rh_ - 1) * PW],
                        in1=xb[:, 2 + (rl + 1) * PW : 2 + (rh_ + 1) * PW],
                        op=ALU.add,
                    )
                    nc.vector.tensor_tensor(
                        out=tv[:, 2 + rl * PW : 2 + rh_ * PW],
                        in0=t_a[:, 2 + rl * PW : 2 + rh_ * PW],
                        in1=xx2[:, rl * PW : rh_ * PW],
                        op=ALU.add,
                    )
                    nc.vector.tensor_tensor(
                        out=ydx[:, (rl - 1) * PW : (rh_ - 1) * PW],
                        in0=tv[:, 3 + rl * PW : 3 + rh_ * PW],
                        in1=tv[:, 1 + rl * PW : 1 + rh_ * PW],
                        op=ALU.subtract,
                    )
                    nc.vector.tensor_tensor(
                        out=ydy[:, (rl - 1) * PW : (rh_ - 1) * PW],
                        in0=th[:, 2 + (rl + 1) * PW : 2 + (rh_ + 1) * PW],
                        in1=th[:, 2 + (rl - 1) * PW : 2 + (rh_ - 1) * PW],
                        op=ALU.subtract,
                    )
                for lo, hi in CAST_BANDS[-1:]:
                    rl, rh_ = max(lo, 1), min(hi, 65)
                    nc.vector.tensor_tensor(
                        out=t_a[:, 2 + rl * PW : 2 + rh_ * PW],
                        in0=xb[:, 2 + (rl - 1) * PW : 2 + (rh_ - 1) * PW],
                        in1=xb[:, 2 + (rl + 1) * PW : 2 + (rh_ + 1) * PW],
                        op=ALU.add,
                    )
                    nc.vector.tensor_tensor(
                        out=tv[:, 2 + rl * PW : 2 + rh_ * PW],
                        in0=t_a[:, 2 + rl * PW : 2 + rh_ * PW],
                        in1=xx2[:, rl * PW : rh_ * PW],
                        op=ALU.add,
                    )
                    nc.vector.tensor_tensor(
                        out=ydx[:, (rl - 1) * PW : (rh_ - 1) * PW],
                        in0=tv[:, 3 + rl * PW : 3 + rh_ * PW],
                        in1=tv[:, 1 + rl * PW : 1 + rh_ * PW],
                        op=ALU.subtract,
                    )
                    nc.vector.tensor_tensor(
                        out=ydy[:, (rl - 1) * PW : (rh_ - 1) * PW],
                        in0=th[:, 2 + (rl + 1) * PW : 2 + (rh_ + 1) * PW],
                        in1=th[:, 2 + (rl - 1) * PW : 2 + (rh_ - 1) * PW],
                        op=ALU.subtract,
                    )

            # --- MLP + residual ---
            xbr = xb[:, 2 : 2 + NPAD].rearrange("p (r w) -> p r w", w=PW)
            ydxr = ydx[:, :].rearrange("p (r w) -> p r w", w=PW)
            ydyr = ydy[:, :].rearrange("p (r w) -> p r w", w=PW)
            xintr = xpad[:, :].rearrange("p (r w) -> p r w", w=PW)

            relu_i = 0
            lp_cm = tc.tile_pool(name="late", bufs=1)
            lp = lp_cm.__enter__()
            x2 = lp.tile([128, NPIX + 2 * X2G], F32, tag="x2")   # data at +X2G
            nc.vector.memset(x2[:, 0:X2G], 0.0)
            nc.vector.memset(x2[:, X2G + NPIX : NPIX + 2 * X2G], 0.0)
            u16 = lp.tile([128, NPIX], BF16, tag="ul", name="u16")
            nc.sync.dma_start(out=u16[:, :], in_=u16_d)
            x2r = x2[:, X2G : X2G + NPIX].rearrange("p (r w) -> p r w", w=W)
            with (
                tc.tile_pool(name="mlp", bufs=1) as mp,
                tc.tile_pool(name="psum", bufs=1, space="PSUM") as psp,
            ):
                prepool = pp.tile([128, 512], F32, tag="prepool")

                def emit_prepool():
                    vm_e = pp.tile([128, 524], F32, tag="vm_e")
                    t1_e = pp.tile([128, 524], F32, tag="t1_e")
                    t2_e = pp.tile([128, 524], F32, tag="t2_e")
                    nc.vector.tensor_tensor(
                        out=t1_e[:, 0:520], in0=astrip[:, 0:520],
                        in1=astrip[:, 130:650], op=ALU.max,
                    )
                    nc.vector.tensor_tensor(
                        out=vm_e[:, 0:520], in0=t1_e[:, 0:520],
                        in1=astrip[:, 260:780], op=ALU.max,
                    )
                    nc.vector.tensor_tensor(
                        out=t2_e[:, 0:519], in0=vm_e[:, 0:519], in1=vm_e[:, 1:520],
                        op=ALU.max,
                    )
                    _vmr = vm_e[:, 0:520].rearrange("p (r w) -> p r w", w=130)
                    _t2r = t2_e[:, 0:520].rearrange("p (r w) -> p r w", w=130)
                    _ppr = prepool[:, :].rearrange("p (r w) -> p r w", w=128)
                    nc.vector.tensor_tensor(
                        out=_ppr[:, 0:4, :], in0=_t2r[:, 0:4, 0:128],
                        in1=_vmr[:, 0:4, 2:130], op=ALU.max,
                    )

                for k in range(NCHUNK):
                    if k == 3:
                        emit_prepool()
                    r0 = 8 * k  # interior row base of chunk
                    dxp = psp.tile([128, CHUNK], F32, tag="dxp", bufs=2)
                    for j in range(4):
                        for gg in range(2):
                            hp = psp.tile([128, CHUNK], F32, tag="hp", bufs=2)
                            for sub in range(2):
                                rr = r0 + 4 * sub
                                hps = hp[:, sub * 512 : sub * 512 + 512]
                                rhss = [
                                    xbr[32 * j : 32 * j + 32, 1 + rr : 5 + rr, 1:129],
                                    ydxr[32 * j : 32 * j + 32, rr : rr + 4, 1:129],
                                    ydyr[32 * j : 32 * j + 32, rr : rr + 4, 1:129],
                                ]
                                for fi, feat in enumerate(("id", "dx", "dy")):
                                    nc.tensor.matmul(
                                        hps,
                                        w0t[(feat, gg)][32 * j : 32 * j + 32, :],
                                        rhss[fi],
                                        start=(fi == 0),
                                        stop=(fi == 2),
                                        tile_position=(32 * j, 0),
                                    )
                            rh = mp.tile([128, CHUNK], BF16, tag="rh", bufs=4)
                            if RELU_PATTERN[relu_i % len(RELU_PATTERN)]:
                                nc.scalar.activation(
                                    out=rh[:, :], in_=hp[:, :], func=ACTF.Relu
                                )
                            else:
                                nc.vector.tensor_scalar_max(rh[:, :], hp[:, :], 0.0)
                            relu_i += 1
                            for sub in range(2):
                                nc.tensor.matmul(
                                    dxp[32 * j : 32 * j + 32, sub * 512 : sub * 512 + 512],
                                    w1t[gg][:, :],
                                    rh[:, sub * 512 : sub * 512 + 512],
                                    start=(gg == 0),
                                    stop=(gg == 1),
                                    tile_position=(0, 32 * j),
                                )
                    st = lp.tile([128, 2048], F32, tag="st", name=f"st{k}", bufs=2)
                    nc.vector.tensor_tensor(
                        out=st[:, 0:CHUNK],
                        in0=dxp[:, :],
                        in1=u16[:, k * CHUNK : (k + 1) * CHUNK],
                        op=ALU.mult,
                    )
                    str_ = st[:, 0:CHUNK].rearrange("p (r w) -> p r w", w=W)
                    nc.vector.tensor_tensor(
                        out=x2r[:, r0 : r0 + 8, :],
                        in0=xintr[:, 1 + r0 : 9 + r0, 1:129],
                        in1=str_,
                        op=ALU.add,
                    )

            # --- alive masks ---
            alp = lp
            if True:
                nc.vector.memset(a2strip[:, :], 0.0)
                # scatter x2 alpha into strip layout: one DMA per halo row r
                PITCH = NPIX + 2 * X2G
                x2ap = x2[:, :]
                a2ap = a2strip[:, :]
                for r in range(6):
                    src = _mk_ap(
                        x2ap, 3 * PITCH + 128 * r,
                        [[16 * PITCH, 8], [512, 16], [1, 128]],
                    )
                    dst = _mk_ap(a2ap, 130 * r + 1, [[780, 128], [1, 128]])
                    nc.sync.dma_start(out=dst, in_=src)
                # cross-half halo rows
                nc.sync.dma_start(
                    out=_mk_ap(a2ap, 15 * 780 + 5 * 130 + 1, [[32 * 780, 4], [1, 128]]),
                    in_=_mk_ap(x2ap, 19 * PITCH + X2G, [[32 * PITCH, 4], [1, 128]]),
                )
                nc.sync.dma_start(
                    out=_mk_ap(a2ap, 16 * 780 + 1, [[32 * 780, 4], [1, 128]]),
                    in_=_mk_ap(
                        x2ap, 3 * PITCH + X2G + 63 * 128, [[32 * PITCH, 4], [1, 128]]
                    ),
                )

                def pool3(src_t, dst_t):
                    vm = alp.tile([128, 524], F32, tag="vm")
                    t1 = alp.tile([128, 524], F32, tag="t1")
                    nc.vector.tensor_tensor(
                        out=t1[:, 0:520], in0=src_t[:, 0:520], in1=src_t[:, 130:650],
                        op=ALU.max,
                    )
                    nc.vector.tensor_tensor(
                        out=vm[:, 0:520], in0=t1[:, 0:520], in1=src_t[:, 260:780],
                        op=ALU.max,
                    )
                    t2 = alp.tile([128, 524], F32, tag="t2")
                    nc.vector.tensor_tensor(
                        out=t2[:, 0:519], in0=vm[:, 0:519], in1=vm[:, 1:520],
                        op=ALU.max,
                    )
                    vmr = vm[:, 0:520].rearrange("p (r w) -> p r w", w=130)
                    t2r = t2[:, 0:520].rearrange("p (r w) -> p r w", w=130)
                    dstr = dst_t[:, :].rearrange("p (r w) -> p r w", w=128)
                    nc.vector.tensor_tensor(
                        out=dstr[:, 0:4, :],
                        in0=t2r[:, 0:4, 0:128],
                        in1=vmr[:, 0:4, 2:130],
                        op=ALU.max,
                    )

                postpool = alp.tile([128, 512], F32, tag="postpool")
                pool3(a2strip, postpool)
                pmin = alp.tile([128, 512], F32, tag="pmin")
                nc.vector.tensor_tensor(
                    out=pmin[:, :], in0=prepool[:, :], in1=postpool[:, :], op=ALU.min
                )
                lifes = alp.tile([128, 512], BF16, tag="lifes")
                nc.vector.tensor_scalar(
                    out=lifes[:, :], in0=pmin[:, :], scalar1=0.1, scalar2=None,
                    op0=ALU.is_gt,
                )

            # --- final mask multiply + store ---
            with tc.tile_pool(name="psum2", bufs=1, space="PSUM") as psp2:
                for k in range(4):
                    lps = psp2.tile([128, 2048], F32, tag="lps", name=f"lps{k}", bufs=2)
                    for tl in range(4):
                        t = 4 * k + tl
                        nc.tensor.matmul(
                            lps[:, 512 * tl : 512 * tl + 512],
                            selt[:, 128 * t : 128 * t + 128],
                            lifes[:, 0:512],
                            start=True,
                            stop=True,
                        )
                    ot = lp.tile([128, 2048], F32, tag="st", name=f"ot{k}", bufs=2)
                    nc.vector.tensor_tensor(
                        out=ot[:, :],
                        in0=x2[:, X2G + 2048 * k : X2G + 2048 * (k + 1)],
                        in1=lps[:, :],
                        op=ALU.mult,
                    )
                    eng = nc.sync if k % 2 == 0 else nc.scalar
                    eng.dma_start(
                        out=out_d[:, 2048 * k : 2048 * (k + 1)], in_=ot[:, :]
                    )

            lp_cm.__exit__(None, None, None)

    _split_multiwaits(nc)
    return nc


def host_prep(x, w0, w1, rand_mask):
    bf = ml_dtypes.bfloat16
    xt = np.ascontiguousarray(x.transpose(0, 3, 1, 2))  # [B, C, H, W]

    xp = np.zeros((B, 2, C, PR, PW), np.float32)
    xp[:, 0, :, 1:66, 1:129] = xt[:, :, 0:65, :]
    xp[:, 1, :, 0:65, 1:129] = xt[:, :, 63:128, :]
    xp = xp.reshape(B, 2, C, NPAD)

    u = (rand_mask[..., 0] <= 0.5).astype(np.float32).reshape(B, 2, 64, W)
    u16 = np.ascontiguousarray(
        np.broadcast_to(u[:, :, None], (B, 2, C, 64, W))
    ).astype(bf).reshape(B, 2, C, NPIX)

    apad = np.zeros((B, H + 2, PW), np.float32)
    apad[:, 1:129, 1:129] = x[..., 3]
    idx = 4 * np.arange(32)[:, None] + np.arange(6)[None, :]
    astr = apad[:, idx, :].reshape(B, 32, 780)  # [B, strip, 6*130]

    W0id = w0[:, 0::3]
    W0dx = w0[:, 1::3] / 8.0
    W0dy = w0[:, 2::3] / 8.0
    w0_arrs = {}
    for feat, Wm in (("id", W0id), ("dx", W0dx), ("dy", W0dy)):
        blk = Wm.T.astype(bf)  # [16 c, 128 o]
        for gg in range(2):
            t = np.zeros((128, 128), bf)
            for j in range(4):
                t[32 * j + 16 * gg : 32 * j + 16 * gg + 16, :] = blk
            w0_arrs[(feat, gg)] = t
    w1_arrs = []
    for gg in range(2):
        t = np.zeros((128, 32), bf)
        t[:, 16 * gg : 16 * gg + 16] = w1.T.astype(bf)
        w1_arrs.append(t)

    sel = np.zeros((128, 2048), bf)
    for t in range(16):
        for p in range(128):
            g = p // 16
            sel[16 * g + t, 128 * t + p] = 1.0

    in_maps = []
    for ci in range(N_CORES):
        sl = slice(IMGS * ci, IMGS * (ci + 1))
        m = {
            "xpad": np.ascontiguousarray(xp[sl]).reshape(128, NPAD),
            "u16": np.ascontiguousarray(u16[sl]).reshape(128, NPIX),
            "astrip": np.ascontiguousarray(astr[sl]).reshape(128, 780),
            "sel": sel,
            "w10": w1_arrs[0],
            "w11": w1_arrs[1],
        }
        for (feat, gg), arr in w0_arrs.items():
            m[f"w0{feat}{gg}"] = arr
        in_maps.append(m)
    return in_maps


def host_post(results):
    out = np.empty((B, H, W, C), np.float32)
    for ci in range(N_CORES):
        o = results[ci]["out"].reshape(IMGS, 2, C, 64, W)
        out[IMGS * ci : IMGS * (ci + 1)] = o.transpose(0, 1, 3, 4, 2).reshape(
            IMGS, H, W, C
        )
    return out


_CACHE = {}


def kernel(x, w0, w1, rand_mask, _trace=False):
    x = np.asarray(x, np.float32)
    w0 = np.asarray(w0, np.float32)
    w1 = np.asarray(w1, np.float32)
    rand_mask = np.asarray(rand_mask, np.float32)

    if "nc" not in _CACHE:
        _CACHE["nc"] = build_program()
    nc = _CACHE["nc"]

    in_maps = host_prep(x, w0, w1, rand_mask)
    res = bass_utils.run_bass_kernel_spmd(
        nc, in_maps, core_ids=list(range(N_CORES)), trace=_trace
    )
    _CACHE["last_result"] = res
    return host_post(res.results)



# revision 11
# speedup vs baseline: 1.9923x; 1.9923x over previous
"""Trainium2 Bass kernel for nn_CAModel (neural cellular automaton step).

v2 — restructured from the 304us baseline around three trace findings:
(1) TensorE was 72% busy on 528 serialized matmul+ldweights pairs,
(2) the PSUM->SBUF relu drain (8.4M elem/core) must be split DVE/ACT,
(3) odd-column-offset conv ops fall off the DVE 2x fast path.

Layout (per core, 4 images): partitions p = (img 4, half 2, chan 16),
free dim = padded half-image rows x 132 pitch (keeps 4B alignment).

- conv: shifted-output formulation so every tensor_tensor op has even
  element offsets (DVE 2x); the x2 scale rides ScalarE's activation
  scale; banded temporaries, ops interleaved between chunk drains.
- fc0: weight-major phases per (feat, half-parity); one replicated
  [128,128] weight serves 4 concurrent row-tiled K=32 matmuls.
- PSUM: two [128,2048] 4-bank tiles ping-pong across half-parities; fc1
  dx accumulates into bank 0 of the first-drained tile (8 banks exact).
- relu drain split ScalarE[0:DA] / VectorE[DA:2048].
- residual + update mask per chunk; alive masks in strip layout; life
  broadcast to channels via PE selector matmuls; bf16 output.
"""

import dataclasses
import numpy as np
import ml_dtypes

import concourse.bass as bass
import concourse.tile as tile
from concourse import mybir, bass_utils

F32 = mybir.dt.float32
BF16 = mybir.dt.bfloat16
ALU = mybir.AluOpType
ACTF = mybir.ActivationFunctionType

N_CORES = 8
B, H, W, C = 32, 128, 128, 16
HID = 128
IMGS = B // N_CORES          # 4 images per core
PW = 132                     # padded row pitch (4B-aligned shifts)
PR = 66                      # padded rows per half (1 + 64 + 1)
NPAD = PR * PW               # 8712
NPIX = 64 * W                # 8192 interior pixels per (img,half)
G = 128                      # guard elems around x2 free dim
PITCH = NPIX + 2 * G         # x2 tile span
NCHUNK = 16                  # chunks of 4 interior rows
CN = 512                     # pixels per (img,half) per chunk

# Tunables
DA = 1536                    # relu drain split: ACT [0:DA], DVE [DA:2048]
YDY_ON_GPSIMD = False         # ydys pass on GpSimd (else DVE)
X2_ON_GPSIMD = False          # residual add on GpSimd (else DVE)


def _split_multiwaits(nc):
    """walrus in this env only supports one sem-wait per instruction."""
    n = 0
    for f in nc.m.functions:
        for bb in f.blocks:
            out = []
            changed = False
            for inst in bb.instructions:
                si = inst.sync_info
                if si is not None and len(si.on_wait) > 1:
                    waits = list(si.on_wait)
                    for k, w in enumerate(waits[:-1]):
                        nop = mybir.InstNoOp(
                            name=f"{inst.name}_ws{k}",
                            sync_info=mybir.SyncInfo(on_wait=[w], on_update=[]),
                            bass_nofuse=True,
                            engine=inst.engine,
                        )
                        nc.register_instruction(nop, overwrite=True)
                        out.append(nop)
                        n += 1
                    inst.sync_info = mybir.SyncInfo(
                        on_wait=[waits[-1]], on_update=list(si.on_update)
                    )
                    changed = True
                out.append(inst)
            if changed:
                bb.instructions[:] = out
    return n


def _mk_ap(ap, offset, dims):
    return dataclasses.replace(ap, offset=offset, ap=[list(d) for d in dims])


def build_program():
    nc = bass.Bass()

    xpad_d = nc.dram_tensor("xpad", [128, NPAD], BF16, kind="ExternalInput").ap()
    u16_d = nc.dram_tensor("u16", [128, NPIX], BF16, kind="ExternalInput").ap()
    astrip_d = nc.dram_tensor("astrip", [128, 780], F32, kind="ExternalInput").ap()
    w0_d = {}
    for feat in ("id", "dx", "dy"):
        for gg in range(2):
            w0_d[(feat, gg)] = nc.dram_tensor(
                f"w0{feat}{gg}", [128, 128], BF16, kind="ExternalInput"
            ).ap()
    w1_d = [
        nc.dram_tensor(f"w1{gg}", [128, 32], BF16, kind="ExternalInput").ap()
        for gg in range(2)
    ]
    sel_d = nc.dram_tensor("sel", [128, 2048], BF16, kind="ExternalInput").ap()
    out_d = nc.dram_tensor("out", [128, NPIX], BF16, kind="ExternalOutput").ap()

    XBANDS = [(0, 8), (8, 19), (19, 30), (30, 41), (41, 52), (52, 66)]

    with tile.TileContext(nc) as tc:
        with (
            tc.tile_pool(name="persist", bufs=1) as pp,
            tc.tile_pool(name="psum", bufs=1, space="PSUM") as psp,
        ):
            xpad = pp.tile([128, NPAD + 4], BF16, tag="xpad")
            u16 = pp.tile([128, NPIX], BF16, tag="u16")
            x2 = pp.tile([128, PITCH], BF16, tag="x2")
            astrip = pp.tile([128, 780], F32, tag="astrip")
            a2strip = pp.tile([128, 780], BF16, tag="a2strip")
            prepool = pp.tile([128, 512], F32, tag="prepool")
            selt = pp.tile([128, 2048], BF16, tag="selt")
            w0t = {
                k: pp.tile([128, 128], BF16, tag=f"w0{k[0]}{k[1]}",
                           name=f"w0t{k[0]}{k[1]}")
                for k in w0_d
            }
            w1t = [
                pp.tile([128, 32], BF16, tag=f"w1{gg}", name=f"w1t{gg}")
                for gg in range(2)
            ]

            # ---- input DMAs (xpad first; bulk on the cheap Pool queue) ----
            for lo, hi in XBANDS:
                nc.sync.dma_start(
                    out=xpad[:, lo * PW : hi * PW], in_=xpad_d[:, lo * PW : hi * PW]
                )
            for k in w0_d:
                nc.gpsimd.dma_start(out=w0t[k][:, :], in_=w0_d[k])
            for gg in range(2):
                nc.gpsimd.dma_start(out=w1t[gg][:, :], in_=w1_d[gg])
            nc.gpsimd.dma_start(out=astrip[:, :], in_=astrip_d)
            nc.gpsimd.dma_start(out=selt[:, :], in_=sel_d)
            for ub in range(4):
                nc.gpsimd.dma_start(
                    out=u16[:, ub * 2048 : (ub + 1) * 2048],
                    in_=u16_d[:, ub * 2048 : (ub + 1) * 2048],
                )

            nc.gpsimd.memset(x2[:, 0:G], 0.0)
            nc.gpsimd.memset(x2[:, G + NPIX : PITCH], 0.0)
            nc.gpsimd.memset(a2strip[:, :], 0.0)

            # ---- conv (shifted-output, all-even offsets) ----
            # th_b[rr,c] = 2*x[pa,c+1] (ACT) then += s_b  == th(pa, c+1)
            # s_b[rr,c]  = x[pa,c] + x[pa,c+2]
            # v_b[rr,c]  = x[pa,c] + x[pa+1,c]
            # tv_b[rr,c] = v[rr,c] + v[rr+1,c]      (true position)
            # yx_b[rr,c] = tv[rr,c+2] - tv[rr,c]    == ydx(., c+1)
            # yy_b[rr,c] = th[rr+2,c] - th[rr,c]    == ydy(., c+1)
            cp = tc.tile_pool(name="conv", bufs=1)
            cpx = cp.__enter__()
            band_tiles = {}

            def alloc_band(b):
                band_tiles[b] = (
                    cpx.tile([128, 18 * PW], BF16, tag="cs", bufs=2,
                             name=f"cs{b}"),
                    cpx.tile([128, 18 * PW], BF16, tag="cth", bufs=2,
                             name=f"cth{b}"),
                    cpx.tile([128, 17 * PW], BF16, tag="cv", bufs=2,
                             name=f"cv{b}"),
                    cpx.tile([128, 16 * PW + 4], BF16, tag="ctv", bufs=2,
                             name=f"ctv{b}"),
                    cpx.tile([128, 16 * PW], BF16, tag="cyx", bufs=2,
                             name=f"cyx{b}"),
                    cpx.tile([128, 16 * PW], BF16, tag="cyy", bufs=2,
                             name=f"cyy{b}"),
                )

            def conv_ops(b, lo, hi):
                """Thunks for interior rows [16b+lo, 16b+hi)."""
                s_b, th_b, v_b, tv_b, yx_b, yy_b = band_tiles[b]
                base = 16 * b
                ydy_eng = nc.gpsimd if YDY_ON_GPSIMD else nc.vector
                return [
                    lambda: nc.scalar.activation(
                        out=th_b[:, lo * PW : (hi + 2) * PW],
                        in_=xpad[:, (base + lo) * PW + 1 : (base + hi + 2) * PW + 1],
                        func=ACTF.Copy, scale=2.0,
                    ),
                    lambda: nc.vector.tensor_tensor(
                        out=s_b[:, lo * PW : (hi + 2) * PW],
                        in0=xpad[:, (base + lo) * PW : (base + hi + 2) * PW],
                        in1=xpad[:, (base + lo) * PW + 2 : (base + hi + 2) * PW + 2],
                        op=ALU.add,
                    ),
                    lambda: nc.vector.tensor_tensor(
                        out=th_b[:, lo * PW : (hi + 2) * PW],
                        in0=th_b[:, lo * PW : (hi + 2) * PW],
                        in1=s_b[:, lo * PW : (hi + 2) * PW],
                        op=ALU.add,
                    ),
                    lambda: nc.vector.tensor_tensor(
                        out=v_b[:, lo * PW : (hi + 1) * PW],
                        in0=xpad[:, (base + lo) * PW : (base + hi + 1) * PW],
                        in1=xpad[:, (base + lo + 1) * PW : (base + hi + 2) * PW],
                        op=ALU.add,
                    ),
                    lambda: nc.vector.tensor_tensor(
                        out=tv_b[:, lo * PW : hi * PW],
                        in0=v_b[:, lo * PW : hi * PW],
                        in1=v_b[:, (lo + 1) * PW : (hi + 1) * PW],
                        op=ALU.add,
                    ),
                    lambda: nc.vector.tensor_tensor(
                        out=yx_b[:, lo * PW : hi * PW],
                        in0=tv_b[:, lo * PW + 2 : hi * PW + 2],
                        in1=tv_b[:, lo * PW : hi * PW],
                        op=ALU.subtract,
                    ),
                    lambda: ydy_eng.tensor_tensor(
                        out=yy_b[:, lo * PW : hi * PW],
                        in0=th_b[:, (lo + 2) * PW : (hi + 2) * PW],
                        in1=th_b[:, lo * PW : hi * PW],
                        op=ALU.subtract,
                    ),
                ]

            # prologue: band 0 in 4-row sub-bands so chunk-0 matmuls start early
            alloc_band(0)
            for sb in range(4):
                for op in conv_ops(0, 4 * sb, 4 * sb + 4):
                    op()
            # band b+1's ops are emitted only during band b's chunks —
            # emitting band b+2 early would head-block the in-order DVE
            # stream on a WAR dep (its buffers are still being read).
            pend = {}
            for b in range(1, 4):
                alloc_band(b)
                pend[b] = conv_ops(b, 0, 16)

            def emit_prepool():
                t1 = pp.tile([128, 524], F32, tag="pp_t1")
                vm = pp.tile([128, 524], F32, tag="pp_vm")
                t2 = pp.tile([128, 524], F32, tag="pp_t2")
                nc.vector.tensor_tensor(
                    out=t1[:, 0:520], in0=astrip[:, 0:520],
                    in1=astrip[:, 130:650], op=ALU.max,
                )
                nc.vector.tensor_tensor(
                    out=vm[:, 0:520], in0=t1[:, 0:520],
                    in1=astrip[:, 260:780], op=ALU.max,
                )
                nc.vector.tensor_tensor(
                    out=t2[:, 0:519], in0=vm[:, 0:519], in1=vm[:, 1:520],
                    op=ALU.max,
                )
                vmr = vm[:, 0:520].rearrange("p (r w) -> p r w", w=130)
                t2r = t2[:, 0:520].rearrange("p (r w) -> p r w", w=130)
                ppr = prepool[:, :].rearrange("p (r w) -> p r w", w=128)
                nc.vector.tensor_tensor(
                    out=ppr[:, 0:4, :], in0=t2r[:, 0:4, 0:128],
                    in1=vmr[:, 0:4, 2:130], op=ALU.max,
                )

            # ---- main chunk loop ----
            A = psp.tile([128, 2048], F32, tag="psA")
            Bp = psp.tile([128, 2048], F32, tag="psB")
            hp = [A, Bp]
            xpr = xpad[:, 0:NPAD].rearrange("p (r w) -> p r w", w=PW)

            for k in range(NCHUNK):
                if k == 2:
                    emit_prepool()
                b = k // 4
                lr0 = 4 * k - 16 * b
                yx_r = band_tiles[b][4][:, :].rearrange("p (r w) -> p r w", w=PW)
                yy_r = band_tiles[b][5][:, :].rearrange("p (r w) -> p r w", w=PW)
                rhss = [
                    xpr[:, 1 + 4 * k : 5 + 4 * k, 2:130],
                    yx_r[:, lr0 : lr0 + 4, 1:129],
                    yy_r[:, lr0 : lr0 + 4, 1:129],
                ]
                Q = hp[k % 2]           # dxp accumulator (bank 0)
                for gg in range(2):
                    P = hp[(k + gg) % 2]
                    for fi, feat in enumerate(("id", "dx", "dy")):
                        for j in range(4):
                            nc.tensor.matmul(
                                P[:, 512 * j : 512 * j + 512],
                                w0t[(feat, gg)][32 * j : 32 * j + 32, :],
                                rhss[fi][32 * j : 32 * j + 32],
                                start=(fi == 0),
                                stop=(fi == 2),
                                tile_position=(32 * j, 0),
                            )
                    rh = pp.tile([128, 2048], BF16, tag=f"rh{gg}", bufs=2)
                    nc.scalar.activation(
                        out=rh[:, 0:DA], in_=P[:, 0:DA], func=ACTF.Relu
                    )
                    nc.vector.tensor_scalar_max(rh[:, DA:2048], P[:, DA:2048], 0.0)
                    for j in range(4):
                        nc.tensor.matmul(
                            Q[32 * j : 32 * j + 32, 0:512],
                            w1t[gg][:, :],
                            rh[:, 512 * j : 512 * j + 512],
                            start=(gg == 0),
                            stop=(gg == 1),
                            tile_position=(0, 32 * j),
                        )
                    nxt = pend.get(b + 1)
                    for _ in range(2):
                        if nxt:
                            nxt.pop(0)()
                st = pp.tile([128, 512], BF16, tag="st", bufs=2)
                nc.vector.tensor_tensor(
                    out=st[:, :],
                    in0=Q[:, 0:512],
                    in1=u16[:, k * CN : (k + 1) * CN],
                    op=ALU.mult,
                )
                x2_eng = nc.gpsimd if X2_ON_GPSIMD else nc.vector
                x2_eng.tensor_tensor(
                    out=x2[:, G + k * CN : G + (k + 1) * CN],
                    in0=xpr[:, 1 + 4 * k : 5 + 4 * k, 2:130],
                    in1=st[:, :],
                    op=ALU.add,
                )

            cpx_exc = cp.__exit__(None, None, None)
            del cpx_exc

            # ---- alive masks ----
            x2ap = x2[:, :]
            a2ap = a2strip[:, :]
            for r in range(6):
                src = _mk_ap(
                    x2ap, 3 * PITCH + 128 * r,
                    [[16 * PITCH, 8], [512, 16], [1, 128]],
                )
                dst = _mk_ap(a2ap, 130 * r + 1, [[780, 128], [1, 128]])
                nc.gpsimd.dma_start(out=dst, in_=src)
            nc.gpsimd.dma_start(
                out=_mk_ap(a2ap, 15 * 780 + 5 * 130 + 1, [[32 * 780, 4], [1, 128]]),
                in_=_mk_ap(x2ap, 19 * PITCH + G, [[32 * PITCH, 4], [1, 128]]),
            )
            nc.gpsimd.dma_start(
                out=_mk_ap(a2ap, 16 * 780 + 1, [[32 * 780, 4], [1, 128]]),
                in_=_mk_ap(
                    x2ap, 3 * PITCH + G + 63 * 128, [[32 * PITCH, 4], [1, 128]]
                ),
            )

            postpool = pp.tile([128, 512], BF16, tag="postpool")
            t1b = pp.tile([128, 524], BF16, tag="ap_t1")
            vmb = pp.tile([128, 524], BF16, tag="ap_vm")
            t2b = pp.tile([128, 524], BF16, tag="ap_t2")
            nc.vector.tensor_tensor(
                out=t1b[:, 0:520], in0=a2strip[:, 0:520],
                in1=a2strip[:, 130:650], op=ALU.max,
            )
            nc.vector.tensor_tensor(
                out=vmb[:, 0:520], in0=t1b[:, 0:520],
                in1=a2strip[:, 260:780], op=ALU.max,
            )
            nc.vector.tensor_tensor(
                out=t2b[:, 0:519], in0=vmb[:, 0:519], in1=vmb[:, 1:520],
                op=ALU.max,
            )
            vmr2 = vmb[:, 0:520].rearrange("p (r w) -> p r w", w=130)
            t2r2 = t2b[:, 0:520].rearrange("p (r w) -> p r w", w=130)
            ppr2 = postpool[:, :].rearrange("p (r w) -> p r w", w=128)
            nc.vector.tensor_tensor(
                out=ppr2[:, 0:4, :], in0=t2r2[:, 0:4, 0:128],
                in1=vmr2[:, 0:4, 2:130], op=ALU.max,
            )
            pmin = pp.tile([128, 512], BF16, tag="pmin")
            nc.vector.tensor_tensor(
                out=pmin[:, :], in0=prepool[:, :], in1=postpool[:, :], op=ALU.min
            )
            lifes = pp.tile([128, 512], BF16, tag="lifes")
            nc.vector.tensor_scalar(
                out=lifes[:, :], in0=pmin[:, :], scalar1=0.1, scalar2=None,
                op0=ALU.is_gt,
            )

            # ---- life broadcast (PE) + final mask multiply + store ----
            for k4 in range(4):
                lps = hp[k4 % 2]
                for tl in range(4):
                    t = 4 * k4 + tl
                    nc.tensor.matmul(
                        lps[:, 512 * tl : 512 * tl + 512],
                        selt[:, 128 * t : 128 * t + 128],
                        lifes[:, 0:512],
                        start=True,
                        stop=True,
                    )
                ot = pp.tile([128, 2048], BF16, tag="ot", bufs=2)
                nc.vector.tensor_tensor(
                    out=ot[:, :],
                    in0=x2[:, G + 2048 * k4 : G + 2048 * (k4 + 1)],
                    in1=lps[:, :],
                    op=ALU.mult,
                )
                nc.sync.dma_start(
                    out=out_d[:, 2048 * k4 : 2048 * (k4 + 1)], in_=ot[:, :]
                )

    _split_multiwaits(nc)
    return nc


def host_prep(x, w0, w1, rand_mask):
    bf = ml_dtypes.bfloat16
    xt = np.ascontiguousarray(x.transpose(0, 3, 1, 2))  # [B, C, H, W]

    xp = np.zeros((B, 2, C, PR, PW), np.float32)
    xp[:, 0, :, 1:66, 2:130] = xt[:, :, 0:65, :]
    xp[:, 1, :, 0:65, 2:130] = xt[:, :, 63:128, :]
    xp = xp.astype(bf).reshape(B, 2, C, NPAD)

    u = (rand_mask[..., 0] <= 0.5).astype(np.float32).reshape(B, 2, 64, W)
    u16 = np.ascontiguousarray(
        np.broadcast_to(u[:, :, None], (B, 2, C, 64, W))
    ).astype(bf).reshape(B, 2, C, NPIX)

    apad = np.zeros((B, H + 2, 130), np.float32)
    apad[:, 1:129, 1:129] = x[..., 3]
    idx = 4 * np.arange(32)[:, None] + np.arange(6)[None, :]
    astr = apad[:, idx, :].reshape(B, 32, 780)  # [B, strip, 6*130]

    W0id = w0[:, 0::3]
    W0dx = w0[:, 1::3] / 8.0
    W0dy = w0[:, 2::3] / 8.0
    w0_arrs = {}
    for feat, Wm in (("id", W0id), ("dx", W0dx), ("dy", W0dy)):
        blk = Wm.T.astype(bf)  # [16 c, 128 o]
        for gg in range(2):
            t = np.zeros((128, 128), bf)
            for j in range(4):
                t[32 * j + 16 * gg : 32 * j + 16 * gg + 16, :] = blk
            w0_arrs[(feat, gg)] = t
    w1_arrs = []
    for gg in range(2):
        t = np.zeros((128, 32), bf)
        t[:, 16 * gg : 16 * gg + 16] = w1.T.astype(bf)
        w1_arrs.append(t)

    sel = np.zeros((128, 2048), bf)
    for t in range(16):
        for p in range(128):
            g = p // 16
            sel[16 * g + t, 128 * t + p] = 1.0

    in_maps = []
    for ci in range(N_CORES):
        sl = slice(IMGS * ci, IMGS * (ci + 1))
        m = {
            "xpad": np.ascontiguousarray(xp[sl]).reshape(128, NPAD),
            "u16": np.ascontiguousarray(u16[sl]).reshape(128, NPIX),
            "astrip": np.ascontiguousarray(astr[sl]).reshape(128, 780),
            "sel": sel,
            "w10": w1_arrs[0],
            "w11": w1_arrs[1],
        }
        for (feat, gg), arr in w0_arrs.items():
            m[f"w0{feat}{gg}"] = arr
        in_maps.append(m)
    return in_maps


def host_post(results):
    out = np.empty((B, H, W, C), np.float32)
    for ci in range(N_CORES):
        o = results[ci]["out"].astype(np.float32).reshape(IMGS, 2, C, 64, W)
        out[IMGS * ci : IMGS * (ci + 1)] = o.transpose(0, 1, 3, 4, 2).reshape(
            IMGS, H, W, C
        )
    return out


_CACHE = {}


def kernel(x, w0, w1, rand_mask, _trace=False):
    x = np.asarray(x, np.float32)
    w0 = np.asarray(w0, np.float32)
    w1 = np.asarray(w1, np.float32)
    rand_mask = np.asarray(rand_mask, np.float32)

    if "nc" not in _CACHE:
        _CACHE["nc"] = build_program()
    nc = _CACHE["nc"]

    in_maps = host_prep(x, w0, w1, rand_mask)
    res = bass_utils.run_bass_kernel_spmd(
        nc, in_maps, core_ids=list(range(N_CORES)), trace=_trace
    )
    _CACHE["last_result"] = res
    return host_post(res.results)
